# revision 22
# baseline (speedup 1.0000x reference)
"""Trainium2 Bass kernel for nn_NUFFTLayerMultiChannel3D_Param_57801669869710.

Factored-NUFFT formulation (no FFTs, everything is matmuls + elementwise):
  The spreading kernel K[n,x,y,z] is separable: gx[n,x]*gy[n,y]*gz[la,z], and
  its (shifted) 3D DFT is the separable product of 1D DFTs ghat.  With
  Ghat_n = fftshift(fftn(K_n)) precomputed on the host (input-independent):

    A: t[c,la,m2]      = sum_lo f[c,la,lo] * Gxy[la,lo,m2]     (Gxy = gxh⊗gyh)
    B: fftv[c,m2,kz]   = sum_la t[c,la,m2] * gzh[la,kz]
       filtered        = fftv * w,  w = deconv * total(params)  (elementwise)
    C: u[c,m2,la]      = sum_kz filtered[c,m2,kz] * conj(gzh[la,kz])
    D: energy[c,la,lo] = (1/N^3) Re sum_m2 u[c,m2,la]*conj(Gxy[la,lo,m2])

  Hermitian symmetry (real input field) keeps only 17 of 32 kz planes with
  paired filter weights wA + wB.

Sharding over 8 cores: la (npoints lat) is split 4-per-core for A/B/C/D; the
partial fftv fields (and per-channel partial filter fields, one channel per
core) are summed with a single fp16 AllReduce; each core then filters and
un-grids its own la slice.  Host gathers the 8 disjoint la slices.

Matmul operands are fp16 (fp32 runs LOW_HIGH double-pass on the PE);
power-of-2 scales folded into the host constants keep everything in fp16
range (the deconv filter reaches ~1e9).  PSUM accumulation is fp32.
"""

import functools

import numpy as np

N = 32
NLAT, NLON = 32, 64
C = 8
NCORES = 8
LAPC = NLAT // NCORES        # la values per core = 4
KZH = 17                     # packed half-space kz planes
M2 = N * N                   # 1024
L = 2.0 * np.pi
TAU = 12.0 * (L / (2.0 * np.pi * N)) ** 2
FFTV_LEN = 34 * 8192         # 278528
TOT_LEN = 34 * M2            # 34816
AR_LEN = FFTV_LEN + 2 * TOT_LEN

SB = 0.25                    # scale folded into BZ (fftv partials)
SW = 2.0 ** -14              # scale folded into wdA/wdB
SG = 2.0 ** 16               # scale folded into GD (undoes SB*SW)


# ----------------------------------------------------------------- host math
@functools.lru_cache(maxsize=1)
def _host_constants():
    lat = np.linspace(-np.pi / 2, np.pi / 2, NLAT)
    lon = np.linspace(0.0, 2.0 * np.pi, NLON)
    la, lo = np.meshgrid(lat, lon, indexing="ij")
    x = np.cos(la) * np.cos(lo)
    y = np.cos(la) * np.sin(lo)
    z = np.sin(lat)
    xg = np.linspace(-np.pi, np.pi, N + 1)[:-1]

    def g(d):
        return (np.exp(-d ** 2 / (4 * TAU))
                + np.exp(-(d - L) ** 2 / (4 * TAU))
                + np.exp(-(d + L) ** 2 / (4 * TAU)))

    gx = g(x[..., None] - xg)                   # (NLAT, NLON, N)
    gy = g(y[..., None] - xg)
    gz = g(z[:, None] - xg)                     # (NLAT, N)

    def sdft(a):
        return np.fft.fftshift(np.fft.fft(a, axis=-1), axes=-1)

    gxh = sdft(gx)
    gyh = sdft(gy)
    gzh = sdft(gz)                              # (NLAT, N) complex

    kg = (2.0 * np.pi / L) * np.linspace(-(N // 2), N // 2, N)
    kx, ky, kz = np.meshgrid(kg, kg, kg, indexing="ij")
    k2 = kx * kx + ky * ky + kz * kz
    kmag = np.sqrt(k2)
    deconv = (np.pi / TAU) ** 1.5 * np.exp(k2 * TAU)

    planes = np.array(list(range(16, 32)) + [0])      # 17 shifted kz planes
    sig = (32 - np.arange(N)) % N                     # shifted-index map for -m

    kmA3 = kmag[:, :, planes]                         # (32, 32, 17)
    decA3 = deconv[:, :, planes]
    kmB3 = kmag[sig][:, sig][:, :, sig][:, :, planes]
    decB3 = deconv[sig][:, sig][:, :, sig][:, :, planes]
    selfp = np.zeros(KZH)
    selfp[0] = 1.0                                    # packed 0  = freq 0
    selfp[16] = 1.0                                   # packed 16 = freq -16
    decB3 = decB3 * (1.0 - selfp)[None, None, :]

    def canon(f3):   # (32ix, 32iy, 17kz) -> flat[(2kz+ri)*1024 + ix*32+iy]
        a = f3.reshape(M2, KZH).T                      # (17, 1024)
        return np.repeat(a[:, None, :], 2, axis=1).reshape(-1)   # (34816,)

    kmA = canon(kmA3).astype(np.float32).reshape(128, 272)
    kmB = canon(kmB3).astype(np.float32).reshape(128, 272)
    wdA = (canon(decA3) * SW).astype(np.float16).reshape(34, 1024)
    wdB = (canon(decB3) * SW).astype(np.float16).reshape(34, 1024)

    gzH = gzh[:, planes]                              # (NLAT, 17) complex

    GA_all, BZ_all, CZ_all, GD_all = [], [], [], []
    for g_ in range(NCORES):
        sl = slice(4 * g_, 4 * g_ + 4)
        Gxy = (gxh[sl][:, :, :, None] * gyh[sl][:, :, None, :]).reshape(4, NLON, M2)
        GRe = Gxy.real.astype(np.float32)
        GIm = Gxy.imag.astype(np.float32)

        # GA2[(lo + 64*lap), pair*2048 + ri*1024 + m2] = RI(Gxy[2*pair+lap])
        GA = np.zeros((128, 4096), np.float16)
        for pair in range(2):
            for lap in range(2):
                la_ = 2 * pair + lap
                GA[64 * lap:64 * lap + 64, pair * 2048:pair * 2048 + 1024] = GRe[la_]
                GA[64 * lap:64 * lap + 64, pair * 2048 + 1024:pair * 2048 + 2048] = GIm[la_]
        GA_all.append(GA)

        gzc = gzH[sl]                                  # (4, 17)
        gzR = (gzc.real * SB).astype(np.float32)
        gzI = (gzc.imag * SB).astype(np.float32)

        bz8 = np.zeros((8, 34), np.float32)            # [2la+ri, 2kz+ri']
        bz8[0::2, 0::2] = gzR
        bz8[1::2, 0::2] = -gzI
        bz8[0::2, 1::2] = gzI
        bz8[1::2, 1::2] = gzR
        BZ = np.zeros((128, 34), np.float16)
        for q in range(4):
            BZ[32 * q:32 * q + 8] = bz8
        BZ_all.append(BZ)

        gzRu = gzc.real.astype(np.float32)
        gzIu = gzc.imag.astype(np.float32)
        cz34 = np.zeros((34, 8), np.float32)           # [2kz+ri, 2la+ri']
        cz34[0::2, 0::2] = gzRu.T
        cz34[1::2, 0::2] = gzIu.T
        cz34[0::2, 1::2] = -gzIu.T
        cz34[1::2, 1::2] = gzRu.T
        CZ = np.zeros((128, 8), np.float16)
        CZ[0:34] = cz34
        CZ[64:98] = cz34
        CZ_all.append(CZ)

        # GD[kp, la*1024 + (ri*8+ctm)*64 + lo] = RI(Gxy[la,lo,kp*8+ctm])*SG/N^3
        GD = np.zeros((128, 4, 2, 8, 64), np.float32)
        scale = SG / (N ** 3)
        GRe5 = (GRe * scale).reshape(4, 64, 128, 8)    # (la, lo, kp, ctm)
        GIm5 = (GIm * scale).reshape(4, 64, 128, 8)
        GD[:, :, 0, :, :] = GRe5.transpose(2, 0, 3, 1)
        GD[:, :, 1, :, :] = GIm5.transpose(2, 0, 3, 1)
        GD_all.append(GD.reshape(128, 4096).astype(np.float16))

    return dict(kmA=kmA, kmB=kmB, wdA=wdA, wdB=wdB,
                GA=GA_all, BZ=BZ_all, CZ=CZ_all, GD=GD_all)


# ------------------------------------------------------------- bass builder
@functools.lru_cache(maxsize=1)
def _build_module():
    import concourse.bass as bass
    import concourse.bacc as bacc
    import concourse.tile as tile
    import concourse.mybir as mybir

    dt32 = mybir.dt.float32
    dt16 = mybir.dt.float16
    AF = mybir.ActivationFunctionType
    MUL = mybir.AluOpType.mult
    ADD = mybir.AluOpType.add
    nc = bacc.Bacc("TRN2", target_bir_lowering=False, debug=False,
                   num_devices=NCORES)

    f_in = nc.dram_tensor("f_in", [128, 32], dt16, kind="ExternalInput").ap()
    prm = nc.dram_tensor("prm", [128, 4], dt32, kind="ExternalInput").ap()
    GA = nc.dram_tensor("GA", [128, 4096], dt16, kind="ExternalInput").ap()
    BZ = nc.dram_tensor("BZ", [128, 34], dt16, kind="ExternalInput").ap()
    CZ = nc.dram_tensor("CZ", [128, 8], dt16, kind="ExternalInput").ap()
    GD = nc.dram_tensor("GD", [128, 4096], dt16, kind="ExternalInput").ap()
    kmA = nc.dram_tensor("kmA", [128, 272], dt32, kind="ExternalInput").ap()
    kmB = nc.dram_tensor("kmB", [128, 272], dt32, kind="ExternalInput").ap()
    wdA = nc.dram_tensor("wdA", [34, 1024], dt16, kind="ExternalInput").ap()
    wdB = nc.dram_tensor("wdB", [34, 1024], dt16, kind="ExternalInput").ap()
    out_e = nc.dram_tensor("out_e", [128, 64], dt32, kind="ExternalOutput").ap()

    with tile.TileContext(nc) as tc:
        with (
            tc.tile_pool(name="sb", bufs=1) as sb,
            tc.tile_pool(name="ps", bufs=8, space="PSUM") as ps,
            tc.tile_pool(name="dr", bufs=1, space="DRAM") as dr,
        ):
            # ---------------- forward-critical loads (A path) first
            s_f = sb.tile([128, 32], dt16)
            nc.sync.dma_start(s_f[:, :], f_in[:, :])
            s_GA = sb.tile([128, 4096], dt16)
            for pair in range(2):
                nc.sync.dma_start(s_GA[:, pair * 2048:(pair + 1) * 2048],
                                  GA[:, pair * 2048:(pair + 1) * 2048])
            s_BZ = sb.tile([128, 34], dt16)
            nc.sync.dma_start(s_BZ[:, :], BZ[:, :])
            s_kmA = sb.tile([128, 272], dt32)
            nc.sync.dma_start(s_kmA[:, :], kmA[:, :])
            s_kmB = sb.tile([128, 272], dt32)
            nc.sync.dma_start(s_kmB[:, :], kmB[:, :])
            s_prm = sb.tile([128, 4], dt32)
            nc.sync.dma_start(s_prm[:, :], prm[:, :])

            d_ARin = dr.tile([AR_LEN], dt16)
            d_ARout = dr.tile([AR_LEN], dt16, addr_space="Shared")
            d_t = dr.tile([65536], dt16)
            d_u = dr.tile([65536], dt16)

            # ---------------- derived params ([128,1] each, fp32)
            p_amp = s_prm[:, 0:1]
            p_sh = s_prm[:, 1:2]
            p_be = s_prm[:, 2:3]
            p_hy = s_prm[:, 3:4]
            s_der = sb.tile([128, 8], dt32)
            d_asq = s_der[:, 0:1]
            d_shsq = s_der[:, 1:2]
            d_s2a = s_der[:, 2:3]
            d_mssh = s_der[:, 3:4]
            d_h400 = s_der[:, 4:5]
            d_bea2 = s_der[:, 5:6]
            d_bea = s_der[:, 6:7]
            nc.vector.tensor_mul(d_asq, p_amp, p_amp)
            nc.vector.tensor_mul(d_shsq, p_sh, p_sh)
            nc.vector.tensor_add(d_s2a, d_asq, d_shsq)
            nc.vector.tensor_scalar_mul(d_mssh, p_sh, -1.0)
            nc.vector.tensor_scalar_mul(d_h400, p_hy, 400.0)
            nc.vector.tensor_mul(d_bea, p_be, p_amp)
            nc.vector.tensor_scalar_mul(d_bea2, d_bea, 2.0)

            def bc(ap):
                return ap.broadcast_to((128, 272))

            # ---------------- total-field partials [128,272] fp32 -> fp16
            # contribution = 2*be*amp*usq*(usq+s2a) / (d1*d2*(usq+400hy)),
            # d1 = (u+sh)^2+amp^2, d2 = (u-sh)^2+amp^2  (no cancellation)
            tot16 = {}
            for F, km in (("A", s_kmA), ("B", s_kmB)):
                d1 = sb.tile([128, 272], dt32, name=f"d1{F}")
                d2 = sb.tile([128, 272], dt32, name=f"d2{F}")
                usq = sb.tile([128, 272], dt32, name=f"usq{F}")
                dd = sb.tile([128, 272], dt32, name=f"dd{F}")
                r = sb.tile([128, 272], dt32, name=f"r{F}")
                e1 = sb.tile([128, 272], dt32, name=f"e1{F}")
                den = sb.tile([128, 272], dt32, name=f"den{F}")
                rec = sb.tile([128, 272], dt32, name=f"rec{F}")
                num = sb.tile([128, 272], dt32, name=f"num{F}")
                tF = sb.tile([128, 272], dt32, name=f"tF{F}")
                nc.scalar.activation(d1, km[:, :], AF.Square, bias=p_sh)
                nc.vector.tensor_tensor(d1, d1, bc(d_asq), ADD)
                nc.scalar.activation(d2, km[:, :], AF.Square, bias=d_mssh)
                nc.gpsimd.tensor_tensor(d2, d2, bc(d_asq), ADD)
                nc.scalar.activation(usq, km[:, :], AF.Square)
                nc.vector.tensor_mul(dd, d1, d2)
                nc.gpsimd.tensor_tensor(r, usq, bc(d_s2a), ADD)
                nc.vector.tensor_tensor(e1, usq, bc(d_h400), ADD)
                nc.vector.tensor_mul(den, dd, e1)
                nc.vector.reciprocal(rec, den)
                nc.gpsimd.tensor_mul(num, usq, r)
                nc.gpsimd.tensor_tensor(num, num, bc(d_bea2), MUL)
                nc.vector.tensor_mul(tF, num, rec)
                tot16[F] = tF

            nc.gpsimd.dma_start(d_ARin[FFTV_LEN:FFTV_LEN + TOT_LEN],
                                tot16["A"][:, :])
            nc.gpsimd.dma_start(d_ARin[FFTV_LEN + TOT_LEN:AR_LEN],
                                tot16["B"][:, :])

            # ---------------- stage A (block-diag la-pairs, K=128, M=16)
            # s_t rows 32*pair + 8*lap + c, free (ri, m2); cast to fp16 on dump
            s_t = sb.tile([64, 2048], dt32)
            for pair in range(2):
                for jj in range(2):
                    psA = ps.tile([16, 1024], dt32, tag="ps2", bufs=2,
                                  name=f"psA{pair}_{jj}")
                    for j2 in range(2):
                        j = 2 * jj + j2
                        nc.tensor.matmul(
                            psA[:, 512 * j2:512 * (j2 + 1)],
                            s_f[:, 16 * pair:16 * pair + 16],
                            s_GA[:, pair * 2048 + 512 * j:
                                 pair * 2048 + 512 * (j + 1)],
                            start=True, stop=True,
                        )
                    eng = nc.vector.tensor_copy if (pair + jj) % 2 == 0 \
                        else nc.scalar.copy
                    eng(s_t[32 * pair:32 * pair + 16,
                            1024 * jj:1024 * (jj + 1)], psA[:, :])

            for la_ in range(4):
                row = 32 * (la_ // 2) + 8 * (la_ % 2)
                nc.gpsimd.dma_start(d_t[la_ * 16384:(la_ + 1) * 16384],
                                    s_t[row:row + 8, :])

            # rB[32q + 2la+ri, cq*1024 + m2], c = 2q+cq
            s_rB = sb.tile([128, 2048], dt16)
            v_t = d_t.rearrange("(la c ri m) -> la ri c m",
                                la=4, c=8, ri=2, m=1024)
            for q in range(4):
                for la_ in range(4):
                    nc.sync.dma_start(
                        s_rB[32 * q + 2 * la_:32 * q + 2 * la_ + 2, :],
                        v_t[la_, :, 2 * q:2 * q + 2, :],
                    )

            # ---------------- stage B: fftv[2kz+ri', c*1024+m2] (row-tiled)
            # per (q): N = 2048 = (cq, m2); psum tiles of 1024 (= one c)
            s_fftv = sb.tile([34, 8192], dt32)
            for q in range(4):
                for cq in range(2):
                    psB = ps.tile([34, 1024], dt32, tag="ps2", bufs=2,
                                  name=f"psB{q}_{cq}")
                    for j2 in range(2):
                        nc.tensor.matmul(
                            psB[:, 512 * j2:512 * (j2 + 1)],
                            s_BZ[32 * q:32 * q + 8, :],
                            s_rB[32 * q:32 * q + 8,
                                 1024 * cq + 512 * j2:1024 * cq + 512 * (j2 + 1)],
                            start=True, stop=True,
                            tile_position=(32 * q, 0),
                        )
                    col = (2 * q + cq) * 1024
                    eng = nc.vector.tensor_copy if (q + cq) % 2 == 0 \
                        else nc.scalar.copy
                    eng(s_fftv[:, col:col + 1024], psB[:, :])

            nc.gpsimd.dma_start(d_ARin[0:FFTV_LEN], s_fftv[:, :])

            # ---------------- AllReduce (fp16 payload)
            nc.gpsimd.collective_compute(
                "AllReduce",
                mybir.AluOpType.add,
                replica_groups=[list(range(NCORES))],
                ins=[d_ARin[:].opt()],
                outs=[d_ARout[:].opt()],
            )

            # backward-only constants (needed post-AR; loaded during AR)
            s_CZ = sb.tile([128, 8], dt16)
            nc.sync.dma_start(s_CZ[:, :], CZ[:, :])
            s_GD = sb.tile([128, 4096], dt16)
            nc.sync.dma_start(s_GD[:, :], GD[:, :])
            s_wdA = sb.tile([128, 1024], dt16)
            s_wdB = sb.tile([128, 1024], dt16)
            for rep in range(2):
                nc.sync.dma_start(s_wdA[64 * rep:64 * rep + 34, :], wdA[:, :])
                nc.sync.dma_start(s_wdB[64 * rep:64 * rep + 34, :], wdB[:, :])

            # ---------------- post-AR loads + filter
            v_fv = d_ARout[0:FFTV_LEN].rearrange("(kr cm) -> kr cm", kr=34)
            v_tA = d_ARout[FFTV_LEN:FFTV_LEN + TOT_LEN].rearrange(
                "(kr m) -> kr m", kr=34)
            v_tB = d_ARout[FFTV_LEN + TOT_LEN:AR_LEN].rearrange(
                "(kr m) -> kr m", kr=34)

            s_fil = sb.tile([128, 4096], dt16)
            s_tsA = sb.tile([128, 1024], dt16)
            s_tsB = sb.tile([128, 1024], dt16)
            s_wt = sb.tile([128, 1024], dt16)
            for rep in range(2):
                sl = slice(64 * rep, 64 * rep + 34)
                nc.sync.dma_start(s_fil[sl, :],
                                  v_fv[:, 4096 * rep:4096 * (rep + 1)])
                nc.sync.dma_start(s_tsA[sl, :], v_tA[:, :])
                nc.sync.dma_start(s_tsB[sl, :], v_tB[:, :])
                nc.vector.tensor_mul(s_wt[sl, :], s_wdA[sl, :], s_tsA[sl, :])
                nc.gpsimd.tensor_mul(s_tsB[sl, :], s_wdB[sl, :], s_tsB[sl, :])
                nc.vector.tensor_add(s_wt[sl, :], s_wt[sl, :], s_tsB[sl, :])
                wt_b = s_wt[sl, :].unsqueeze(1).broadcast_to((34, 4, 1024))
                fil3 = s_fil[sl, :].rearrange("p (c m) -> p c m", c=4)
                if rep == 0:
                    nc.vector.tensor_tensor(fil3, fil3, wt_b, MUL)
                else:
                    nc.gpsimd.tensor_tensor(fil3, fil3, wt_b, MUL)

            # ---------------- stage C (2 rep row-tiles)
            s_u2 = sb.tile([8, 8192], dt32)
            v_u2 = s_u2.rearrange("p (m c) -> p m c", m=1024, c=8)
            for rep in range(2):
                sl = slice(64 * rep, 64 * rep + 34)
                for jc in range(4):          # one c per psum tile
                    psC = ps.tile([8, 1024], dt32, tag="ps2", bufs=2,
                                  name=f"psC{rep}_{jc}")
                    for j2 in range(2):
                        nc.tensor.matmul(
                            psC[:, 512 * j2:512 * (j2 + 1)],
                            s_CZ[sl, :],
                            s_fil[sl, 1024 * jc + 512 * j2:
                                  1024 * jc + 512 * (j2 + 1)],
                            start=True, stop=True,
                        )
                    c_ = 4 * rep + jc
                    dst = v_u2[:, :, c_]
                    eng = nc.vector.tensor_copy if (rep + jc) % 2 == 0 \
                        else nc.scalar.copy
                    eng(dst, psC[:, :])

            nc.gpsimd.dma_start(d_u[0:65536], s_u2[:, :])

            # reload for D: uD[kp, la*128 + (ri*8+ctm)*8 + c]
            s_uD = sb.tile([128, 512], dt16)
            v_uD = s_uD.rearrange("p (la ct c) -> p la ct c", la=4, ct=16, c=8)
            v_du = d_u.rearrange("(la ri kp ctm c) -> kp la ri ctm c",
                                 la=4, ri=2, kp=128, ctm=8, c=8)
            for ri in range(2):
                nc.sync.dma_start(
                    v_uD[:, :, 8 * ri:8 * ri + 8, :],
                    v_du[:, :, ri, :, :],
                )

            # ---------------- stage D (4 la col-tiled, 16-chunk accumulate)
            s_out = sb.tile([128, 64], dt32)
            psD = [ps.tile([128, 64], dt32, tag="psd", bufs=4,
                           name=f"psD{la_}")
                   for la_ in range(4)]
            for ct in range(16):
                for la_ in range(4):
                    nc.tensor.matmul(
                        psD[la_][32 * la_:32 * la_ + 8, :],
                        s_uD[:, la_ * 128 + ct * 8:la_ * 128 + ct * 8 + 8],
                        s_GD[:, la_ * 1024 + ct * 64:la_ * 1024 + ct * 64 + 64],
                        start=(ct == 0), stop=(ct == 15),
                        tile_position=(0, 32 * la_),
                    )
            for la_ in range(4):
                eng_copy = (nc.vector.tensor_copy if la_ % 2 == 0
                            else nc.scalar.copy)
                eng_copy(s_out[32 * la_:32 * la_ + 8, :],
                         psD[la_][32 * la_:32 * la_ + 8, :])

            for la_ in range(4):
                nc.sync.dma_start(out_e[32 * la_:32 * la_ + 8, :],
                                  s_out[32 * la_:32 * la_ + 8, :])

    nc.compile()
    return nc


def _make_in_maps(inp, amplitude, shift, beta, hypera):
    consts = _host_constants()
    inp = np.ascontiguousarray(np.asarray(inp, np.float32))
    prms = [np.asarray(a, np.float32).reshape(-1) for a in
            (amplitude, shift, beta, hypera)]
    in_maps = []
    for g_ in range(NCORES):
        # block-diag f: [lo + 64*lap, pair*16 + 8*lap' + c], nonzero lap'==lap
        f = np.zeros((128, 32), np.float16)
        for pair in range(2):
            for lap in range(2):
                la_ = 2 * pair + lap
                f[64 * lap:64 * lap + 64, pair * 16 + 8 * lap:pair * 16 + 8 * lap + 8] = \
                    inp[0, :, 4 * g_ + la_, :].T
        prm = np.zeros((128, 4), np.float32)
        prm[:, 0] = prms[0][g_]
        prm[:, 1] = prms[1][g_]
        prm[:, 2] = prms[2][g_]
        prm[:, 3] = prms[3][g_]
        in_maps.append({
            "f_in": f,
            "prm": prm,
            "GA": consts["GA"][g_],
            "BZ": consts["BZ"][g_],
            "CZ": consts["CZ"][g_],
            "GD": consts["GD"][g_],
            "kmA": consts["kmA"],
            "kmB": consts["kmB"],
            "wdA": consts["wdA"],
            "wdB": consts["wdB"],
        })
    return in_maps


def _assemble(outs):
    energy = np.zeros((C, NLAT, NLON), np.float32)
    for g_ in range(NCORES):
        oe = outs[g_]["out_e"]                  # (128, 64)
        for la_ in range(LAPC):
            energy[:, 4 * g_ + la_, :] = oe[32 * la_:32 * la_ + 8, :]
    pred = energy.reshape(C, NLAT * NLON).T.reshape(1, C, NLAT, NLON)
    return pred


def kernel(inp, amplitude, shift, beta, hypera, _trace=False):
    from concourse.bass_utils import run_bass_kernel_spmd

    nc = _build_module()
    in_maps = _make_in_maps(inp, amplitude, shift, beta, hypera)
    res = run_bass_kernel_spmd(nc, in_maps, core_ids=list(range(NCORES)),
                               trace=_trace)
    out = _assemble(res.results)
    if _trace:
        kernel.last_results = res
    return out


# revision 30
# speedup vs baseline: 1.0301x; 1.0301x over previous
"""Trainium2 Bass kernel for nn_NUFFTLayerMultiChannel3D_Param_57801669869710.

Factored-NUFFT formulation (no FFTs, everything is matmuls + elementwise):
  The spreading kernel K[n,x,y,z] is separable: gx[n,x]*gy[n,y]*gz[la,z], and
  its (shifted) 3D DFT is the separable product of 1D DFTs ghat.  With
  Ghat_n = fftshift(fftn(K_n)) precomputed on the host (input-independent):

    A: t[c,la,m2]      = sum_lo f[c,la,lo] * Gxy[la,lo,m2]     (Gxy = gxh⊗gyh)
    B: fftv[c,m2,kz]   = sum_la t[c,la,m2] * gzh[la,kz]
       filtered        = fftv * w,  w = deconv * total(params)  (elementwise)
    C: u[c,m2,la]      = sum_kz filtered[c,m2,kz] * conj(gzh[la,kz])
    D: energy[c,la,lo] = (1/N^3) Re sum_m2 u[c,m2,la]*conj(Gxy[la,lo,m2])

  Hermitian symmetry (real input field) keeps only 17 of 32 kz planes with
  paired filter weights wA + wB.

Sharding over 8 cores: la (npoints lat) is split 4-per-core for A/B/C/D; the
partial fftv fields (and per-channel partial filter fields, one channel per
core) are summed with a single fp16 AllReduce; each core then filters and
un-grids its own la slice.  Host gathers the 8 disjoint la slices.

Matmul operands are fp16 (fp32 runs LOW_HIGH double-pass on the PE);
power-of-2 scales folded into the host constants keep everything in fp16
range (the deconv filter reaches ~1e9).  PSUM accumulation is fp32.
"""

import functools

import numpy as np

N = 32
NLAT, NLON = 32, 64
C = 8
NCORES = 8
LAPC = NLAT // NCORES        # la values per core = 4
KZH = 17                     # packed half-space kz planes
M2 = N * N                   # 1024
L = 2.0 * np.pi
TAU = 12.0 * (L / (2.0 * np.pi * N)) ** 2
FFTV_LEN = 34 * 8192         # 278528
TOT_LEN = 34 * M2            # 34816
AR_LEN = FFTV_LEN + 2 * TOT_LEN

SB = 0.25                    # scale folded into BZ (fftv partials)
SW = 2.0 ** -14              # scale folded into wdA/wdB
SG = 2.0 ** 16               # scale folded into GD (undoes SB*SW)


# ----------------------------------------------------------------- host math
@functools.lru_cache(maxsize=1)
def _host_constants():
    lat = np.linspace(-np.pi / 2, np.pi / 2, NLAT)
    lon = np.linspace(0.0, 2.0 * np.pi, NLON)
    la, lo = np.meshgrid(lat, lon, indexing="ij")
    x = np.cos(la) * np.cos(lo)
    y = np.cos(la) * np.sin(lo)
    z = np.sin(lat)
    xg = np.linspace(-np.pi, np.pi, N + 1)[:-1]

    def g(d):
        return (np.exp(-d ** 2 / (4 * TAU))
                + np.exp(-(d - L) ** 2 / (4 * TAU))
                + np.exp(-(d + L) ** 2 / (4 * TAU)))

    gx = g(x[..., None] - xg)                   # (NLAT, NLON, N)
    gy = g(y[..., None] - xg)
    gz = g(z[:, None] - xg)                     # (NLAT, N)

    def sdft(a):
        return np.fft.fftshift(np.fft.fft(a, axis=-1), axes=-1)

    gxh = sdft(gx)
    gyh = sdft(gy)
    gzh = sdft(gz)                              # (NLAT, N) complex

    kg = (2.0 * np.pi / L) * np.linspace(-(N // 2), N // 2, N)
    kx, ky, kz = np.meshgrid(kg, kg, kg, indexing="ij")
    k2 = kx * kx + ky * ky + kz * kz
    kmag = np.sqrt(k2)
    deconv = (np.pi / TAU) ** 1.5 * np.exp(k2 * TAU)

    planes = np.array(list(range(16, 32)) + [0])      # 17 shifted kz planes
    sig = (32 - np.arange(N)) % N                     # shifted-index map for -m

    kmA3 = kmag[:, :, planes]                         # (32, 32, 17)
    decA3 = deconv[:, :, planes]
    kmB3 = kmag[sig][:, sig][:, :, sig][:, :, planes]
    decB3 = deconv[sig][:, sig][:, :, sig][:, :, planes]
    selfp = np.zeros(KZH)
    selfp[0] = 1.0                                    # packed 0  = freq 0
    selfp[16] = 1.0                                   # packed 16 = freq -16
    decB3 = decB3 * (1.0 - selfp)[None, None, :]

    def canon(f3):   # (32ix, 32iy, 17kz) -> flat[(2kz+ri)*1024 + ix*32+iy]
        a = f3.reshape(M2, KZH).T                      # (17, 1024)
        return np.repeat(a[:, None, :], 2, axis=1).reshape(-1)   # (34816,)

    kmA = canon(kmA3).astype(np.float32).reshape(128, 272)
    kmB = canon(kmB3).astype(np.float32).reshape(128, 272)
    wdA = (canon(decA3) * SW).astype(np.float16).reshape(34, 1024)
    wdB = (canon(decB3) * SW).astype(np.float16).reshape(34, 1024)

    gzH = gzh[:, planes]                              # (NLAT, 17) complex

    GA_all, BZ_all, CZ_all, GD_all = [], [], [], []
    for g_ in range(NCORES):
        sl = slice(4 * g_, 4 * g_ + 4)
        Gxy = (gxh[sl][:, :, :, None] * gyh[sl][:, :, None, :]).reshape(4, NLON, M2)
        GRe = Gxy.real.astype(np.float32)
        GIm = Gxy.imag.astype(np.float32)

        # GA2[(lo + 64*lap), pair*2048 + ri*1024 + m2] = RI(Gxy[2*pair+lap])
        GA = np.zeros((128, 4096), np.float16)
        for pair in range(2):
            for lap in range(2):
                la_ = 2 * pair + lap
                GA[64 * lap:64 * lap + 64, pair * 2048:pair * 2048 + 1024] = GRe[la_]
                GA[64 * lap:64 * lap + 64, pair * 2048 + 1024:pair * 2048 + 2048] = GIm[la_]
        GA_all.append(GA)

        gzc = gzH[sl]                                  # (4, 17)
        gzR = (gzc.real * SB).astype(np.float32)
        gzI = (gzc.imag * SB).astype(np.float32)

        bz8 = np.zeros((8, 34), np.float32)            # [2la+ri, 2kz+ri']
        bz8[0::2, 0::2] = gzR
        bz8[1::2, 0::2] = -gzI
        bz8[0::2, 1::2] = gzI
        bz8[1::2, 1::2] = gzR
        BZ = np.zeros((128, 34), np.float16)
        for q in range(4):
            BZ[32 * q:32 * q + 8] = bz8
        BZ_all.append(BZ)

        gzRu = gzc.real.astype(np.float32)
        gzIu = gzc.imag.astype(np.float32)
        cz34 = np.zeros((34, 8), np.float32)           # [2kz+ri, 2la+ri']
        cz34[0::2, 0::2] = gzRu.T
        cz34[1::2, 0::2] = gzIu.T
        cz34[0::2, 1::2] = -gzIu.T
        cz34[1::2, 1::2] = gzRu.T
        CZ = np.zeros((128, 8), np.float16)
        CZ[0:34] = cz34
        CZ[64:98] = cz34
        CZ_all.append(CZ)

        # GD[kp, la*1024 + (ri*8+ctm)*64 + lo] = RI(Gxy[la,lo,kp*8+ctm])*SG/N^3
        GD = np.zeros((128, 4, 2, 8, 64), np.float32)
        scale = SG / (N ** 3)
        GRe5 = (GRe * scale).reshape(4, 64, 128, 8)    # (la, lo, kp, ctm)
        GIm5 = (GIm * scale).reshape(4, 64, 128, 8)
        GD[:, :, 0, :, :] = GRe5.transpose(2, 0, 3, 1)
        GD[:, :, 1, :, :] = GIm5.transpose(2, 0, 3, 1)
        GD_all.append(GD.reshape(128, 4096).astype(np.float16))

    return dict(kmA=kmA, kmB=kmB, wdA=wdA, wdB=wdB,
                GA=GA_all, BZ=BZ_all, CZ=CZ_all, GD=GD_all)


# ------------------------------------------------------------- bass builder
@functools.lru_cache(maxsize=1)
def _build_module():
    import concourse.bass as bass
    import concourse.bacc as bacc
    import concourse.tile as tile
    import concourse.mybir as mybir

    dt32 = mybir.dt.float32
    dt16 = mybir.dt.float16
    AF = mybir.ActivationFunctionType
    MUL = mybir.AluOpType.mult
    ADD = mybir.AluOpType.add
    nc = bacc.Bacc("TRN2", target_bir_lowering=False, debug=False,
                   num_devices=NCORES)

    f_in = nc.dram_tensor("f_in", [128, 32], dt16, kind="ExternalInput").ap()
    prm = nc.dram_tensor("prm", [128, 4], dt32, kind="ExternalInput").ap()
    GA = nc.dram_tensor("GA", [128, 4096], dt16, kind="ExternalInput").ap()
    BZ = nc.dram_tensor("BZ", [128, 34], dt16, kind="ExternalInput").ap()
    CZ = nc.dram_tensor("CZ", [128, 8], dt16, kind="ExternalInput").ap()
    GD = nc.dram_tensor("GD", [128, 4096], dt16, kind="ExternalInput").ap()
    kmA = nc.dram_tensor("kmA", [128, 272], dt32, kind="ExternalInput").ap()
    kmB = nc.dram_tensor("kmB", [128, 272], dt32, kind="ExternalInput").ap()
    wdA = nc.dram_tensor("wdA", [34, 1024], dt16, kind="ExternalInput").ap()
    wdB = nc.dram_tensor("wdB", [34, 1024], dt16, kind="ExternalInput").ap()
    out_e = nc.dram_tensor("out_e", [128, 64], dt32, kind="ExternalOutput").ap()

    with tile.TileContext(nc) as tc:
        with (
            tc.tile_pool(name="sb", bufs=1) as sb,
            tc.tile_pool(name="ps", bufs=8, space="PSUM") as ps,
            tc.tile_pool(name="dr", bufs=1, space="DRAM") as dr,
        ):
            # ---------------- forward-critical loads (A path) first
            s_f = sb.tile([128, 32], dt16)
            nc.sync.dma_start(s_f[:, :], f_in[:, :])
            s_GA = sb.tile([128, 4096], dt16)
            for pair in range(2):
                nc.sync.dma_start(s_GA[:, pair * 2048:(pair + 1) * 2048],
                                  GA[:, pair * 2048:(pair + 1) * 2048])
            s_BZ = sb.tile([128, 34], dt16)
            nc.sync.dma_start(s_BZ[:, :], BZ[:, :])
            s_kmA = sb.tile([128, 272], dt32)
            nc.sync.dma_start(s_kmA[:, :], kmA[:, :])
            s_kmB = sb.tile([128, 272], dt32)
            nc.sync.dma_start(s_kmB[:, :], kmB[:, :])
            s_prm = sb.tile([128, 4], dt32)
            nc.sync.dma_start(s_prm[:, :], prm[:, :])

            d_ARin = dr.tile([AR_LEN], dt16)
            d_ARout = dr.tile([AR_LEN], dt16, addr_space="Shared")
            d_t = dr.tile([65536], dt16)
            d_u = dr.tile([65536], dt16)

            # ---------------- derived params ([128,1] each, fp32)
            p_amp = s_prm[:, 0:1]
            p_sh = s_prm[:, 1:2]
            p_be = s_prm[:, 2:3]
            p_hy = s_prm[:, 3:4]
            s_der = sb.tile([128, 8], dt32)
            d_asq = s_der[:, 0:1]
            d_shsq = s_der[:, 1:2]
            d_s2a = s_der[:, 2:3]
            d_mssh = s_der[:, 3:4]
            d_h400 = s_der[:, 4:5]
            d_bea2 = s_der[:, 5:6]
            d_bea = s_der[:, 6:7]
            nc.vector.tensor_mul(d_asq, p_amp, p_amp)
            nc.vector.tensor_mul(d_shsq, p_sh, p_sh)
            nc.vector.tensor_add(d_s2a, d_asq, d_shsq)
            nc.vector.tensor_scalar_mul(d_mssh, p_sh, -1.0)
            nc.vector.tensor_scalar_mul(d_h400, p_hy, 400.0)
            nc.vector.tensor_mul(d_bea, p_be, p_amp)
            nc.vector.tensor_scalar_mul(d_bea2, d_bea, 2.0)

            def bc(ap):
                return ap.broadcast_to((128, 272))

            # ---------------- total-field partials [128,272] fp32 -> fp16
            # contribution = 2*be*amp*usq*(usq+s2a) / (d1*d2*(usq+400hy)),
            # d1 = (u+sh)^2+amp^2, d2 = (u-sh)^2+amp^2  (no cancellation)
            tot16 = {}
            for F, km in (("A", s_kmA), ("B", s_kmB)):
                d1 = sb.tile([128, 272], dt32, name=f"d1{F}")
                d2 = sb.tile([128, 272], dt32, name=f"d2{F}")
                usq = sb.tile([128, 272], dt32, name=f"usq{F}")
                dd = sb.tile([128, 272], dt32, name=f"dd{F}")
                r = sb.tile([128, 272], dt32, name=f"r{F}")
                e1 = sb.tile([128, 272], dt32, name=f"e1{F}")
                den = sb.tile([128, 272], dt32, name=f"den{F}")
                rec = sb.tile([128, 272], dt32, name=f"rec{F}")
                num = sb.tile([128, 272], dt32, name=f"num{F}")
                tF = sb.tile([128, 272], dt32, name=f"tF{F}")
                nc.scalar.activation(d1, km[:, :], AF.Square, bias=p_sh)
                nc.vector.tensor_tensor(d1, d1, bc(d_asq), ADD)
                nc.scalar.activation(d2, km[:, :], AF.Square, bias=d_mssh)
                nc.vector.tensor_tensor(d2, d2, bc(d_asq), ADD)
                nc.scalar.activation(usq, km[:, :], AF.Square)
                nc.vector.tensor_mul(dd, d1, d2)
                nc.vector.tensor_tensor(r, usq, bc(d_s2a), ADD)
                nc.vector.tensor_tensor(e1, usq, bc(d_h400), ADD)
                nc.vector.tensor_mul(den, dd, e1)
                nc.vector.reciprocal(rec, den)
                nc.vector.tensor_mul(num, usq, r)
                nc.vector.tensor_tensor(num, num, bc(d_bea2), MUL)
                nc.vector.tensor_mul(tF, num, rec)
                tot16[F] = tF

            nc.gpsimd.dma_start(d_ARin[FFTV_LEN:FFTV_LEN + TOT_LEN],
                                tot16["A"][:, :])
            nc.gpsimd.dma_start(d_ARin[FFTV_LEN + TOT_LEN:AR_LEN],
                                tot16["B"][:, :])

            # ---------------- stage A (block-diag la-pairs, K=128, M=16)
            # s_t rows 32*pair + 8*lap + c, free (ri, m2); cast to fp16 on dump
            s_t = sb.tile([64, 2048], dt32)
            for pair in range(2):
                for jj in range(2):
                    psA = ps.tile([16, 1024], dt32, tag="ps2", bufs=2,
                                  name=f"psA{pair}_{jj}")
                    for j2 in range(2):
                        j = 2 * jj + j2
                        nc.tensor.matmul(
                            psA[:, 512 * j2:512 * (j2 + 1)],
                            s_f[:, 16 * pair:16 * pair + 16],
                            s_GA[:, pair * 2048 + 512 * j:
                                 pair * 2048 + 512 * (j + 1)],
                            start=True, stop=True,
                        )
                    eng = nc.vector.tensor_copy if (pair + jj) % 2 == 0 \
                        else nc.scalar.copy
                    eng(s_t[32 * pair:32 * pair + 16,
                            1024 * jj:1024 * (jj + 1)], psA[:, :])

            for la_ in range(4):
                row = 32 * (la_ // 2) + 8 * (la_ % 2)
                nc.gpsimd.dma_start(d_t[la_ * 16384:(la_ + 1) * 16384],
                                    s_t[row:row + 8, :])

            # rB[32q + 2la+ri, cq*1024 + m2], c = 2q+cq
            s_rB = sb.tile([128, 2048], dt16)
            v_t = d_t.rearrange("(la c ri m) -> ri la c m",
                                la=4, c=8, ri=2, m=1024)
            for q in range(4):
                for ri in range(2):
                    # partitions 32q+ri, step 2 over la
                    dst = s_rB[32 * q + ri:32 * q + ri + 7:2, :]
                    nc.sync.dma_start(dst, v_t[ri, :, 2 * q:2 * q + 2, :])

            # ---------------- stage B: fftv[2kz+ri', c*1024+m2] (row-tiled)
            # per (q): N = 2048 = (cq, m2); psum tiles of 1024 (= one c)
            s_fftv = sb.tile([34, 8192], dt32)
            for cq in range(2):
                for q in range(4):
                    psB = ps.tile([34, 1024], dt32, tag="ps2", bufs=2,
                                  name=f"psB{q}_{cq}")
                    for j2 in range(2):
                        nc.tensor.matmul(
                            psB[:, 512 * j2:512 * (j2 + 1)],
                            s_BZ[32 * q:32 * q + 8, :],
                            s_rB[32 * q:32 * q + 8,
                                 1024 * cq + 512 * j2:1024 * cq + 512 * (j2 + 1)],
                            start=True, stop=True,
                            tile_position=(32 * q, 0),
                        )
                    col = (2 * q + cq) * 1024
                    eng = nc.vector.tensor_copy if (q + cq) % 2 == 0 \
                        else nc.scalar.copy
                    eng(s_fftv[:, col:col + 1024], psB[:, :])

            nc.gpsimd.dma_start(d_ARin[0:FFTV_LEN], s_fftv[:, :])

            # ---------------- AllReduce (fp16 payload)
            nc.gpsimd.collective_compute(
                "AllReduce",
                mybir.AluOpType.add,
                replica_groups=[list(range(NCORES))],
                ins=[d_ARin[:].opt()],
                outs=[d_ARout[:].opt()],
            )

            # backward-only constants (needed post-AR; loaded during AR)
            s_CZ = sb.tile([128, 8], dt16)
            nc.sync.dma_start(s_CZ[:, :], CZ[:, :])
            s_GD = sb.tile([128, 4096], dt16)
            nc.sync.dma_start(s_GD[:, :], GD[:, :])
            s_wdA = sb.tile([128, 1024], dt16)
            s_wdB = sb.tile([128, 1024], dt16)
            for rep in range(2):
                nc.sync.dma_start(s_wdA[64 * rep:64 * rep + 34, :], wdA[:, :])
                nc.sync.dma_start(s_wdB[64 * rep:64 * rep + 34, :], wdB[:, :])

            # ---------------- post-AR loads + filter
            v_fv = d_ARout[0:FFTV_LEN].rearrange("(kr cm) -> kr cm", kr=34)
            v_tA = d_ARout[FFTV_LEN:FFTV_LEN + TOT_LEN].rearrange(
                "(kr m) -> kr m", kr=34)
            v_tB = d_ARout[FFTV_LEN + TOT_LEN:AR_LEN].rearrange(
                "(kr m) -> kr m", kr=34)

            s_fil = sb.tile([128, 4096], dt16)
            s_tsA = sb.tile([128, 1024], dt16)
            s_tsB = sb.tile([128, 1024], dt16)
            s_wt = sb.tile([128, 1024], dt16)
            for rep in range(2):
                sl = slice(64 * rep, 64 * rep + 34)
                nc.sync.dma_start(s_fil[sl, :],
                                  v_fv[:, 4096 * rep:4096 * (rep + 1)])
                nc.sync.dma_start(s_tsA[sl, :], v_tA[:, :])
                nc.sync.dma_start(s_tsB[sl, :], v_tB[:, :])
                nc.vector.tensor_mul(s_wt[sl, :], s_wdA[sl, :], s_tsA[sl, :])
                nc.vector.tensor_mul(s_tsB[sl, :], s_wdB[sl, :], s_tsB[sl, :])
                nc.vector.tensor_add(s_wt[sl, :], s_wt[sl, :], s_tsB[sl, :])
                wt_b = s_wt[sl, :].unsqueeze(1).broadcast_to((34, 4, 1024))
                fil3 = s_fil[sl, :].rearrange("p (c m) -> p c m", c=4)
                nc.vector.tensor_tensor(fil3, fil3, wt_b, MUL)

            # ---------------- stage C (2 rep row-tiles)
            s_u2 = sb.tile([8, 8192], dt32)
            v_u2 = s_u2.rearrange("p (m c) -> p m c", m=1024, c=8)
            for jc in range(4):              # one c per psum tile
                for rep in range(2):
                    sl = slice(64 * rep, 64 * rep + 34)
                    psC = ps.tile([8, 1024], dt32, tag="ps2", bufs=2,
                                  name=f"psC{rep}_{jc}")
                    for j2 in range(2):
                        nc.tensor.matmul(
                            psC[:, 512 * j2:512 * (j2 + 1)],
                            s_CZ[sl, :],
                            s_fil[sl, 1024 * jc + 512 * j2:
                                  1024 * jc + 512 * (j2 + 1)],
                            start=True, stop=True,
                        )
                    c_ = 4 * rep + jc
                    dst = v_u2[:, :, c_]
                    eng = nc.vector.tensor_copy if (rep + jc) % 2 == 0 \
                        else nc.scalar.copy
                    eng(dst, psC[:, :])

            nc.gpsimd.dma_start(d_u[0:65536], s_u2[:, :])

            # reload for D: uD[kp, la*128 + (ri*8+ctm)*8 + c]
            s_uD = sb.tile([128, 512], dt16)
            v_uD = s_uD.rearrange("p (la ct c) -> p la ct c", la=4, ct=16, c=8)
            v_du = d_u.rearrange("(la ri kp ctm c) -> kp la ri ctm c",
                                 la=4, ri=2, kp=128, ctm=8, c=8)
            for ri in range(2):
                nc.sync.dma_start(
                    v_uD[:, :, 8 * ri:8 * ri + 8, :],
                    v_du[:, :, ri, :, :],
                )

            # ---------------- stage D (4 la col-tiled, 16-chunk accumulate)
            s_out = sb.tile([128, 64], dt32)
            psD = [ps.tile([128, 64], dt32, tag="psd", bufs=4,
                           name=f"psD{la_}")
                   for la_ in range(4)]
            for ct in range(16):
                for la_ in range(4):
                    nc.tensor.matmul(
                        psD[la_][32 * la_:32 * la_ + 8, :],
                        s_uD[:, la_ * 128 + ct * 8:la_ * 128 + ct * 8 + 8],
                        s_GD[:, la_ * 1024 + ct * 64:la_ * 1024 + ct * 64 + 64],
                        start=(ct == 0), stop=(ct == 15),
                        tile_position=(0, 32 * la_),
                    )
            for la_ in range(4):
                eng_copy = (nc.vector.tensor_copy if la_ % 2 == 0
                            else nc.scalar.copy)
                eng_copy(s_out[32 * la_:32 * la_ + 8, :],
                         psD[la_][32 * la_:32 * la_ + 8, :])

            for la_ in range(4):
                nc.sync.dma_start(out_e[32 * la_:32 * la_ + 8, :],
                                  s_out[32 * la_:32 * la_ + 8, :])

    nc.compile()
    return nc


def _make_in_maps(inp, amplitude, shift, beta, hypera):
    consts = _host_constants()
    inp = np.ascontiguousarray(np.asarray(inp, np.float32))
    prms = [np.asarray(a, np.float32).reshape(-1) for a in
            (amplitude, shift, beta, hypera)]
    in_maps = []
    for g_ in range(NCORES):
        # block-diag f: [lo + 64*lap, pair*16 + 8*lap' + c], nonzero lap'==lap
        f = np.zeros((128, 32), np.float16)
        for pair in range(2):
            for lap in range(2):
                la_ = 2 * pair + lap
                f[64 * lap:64 * lap + 64, pair * 16 + 8 * lap:pair * 16 + 8 * lap + 8] = \
                    inp[0, :, 4 * g_ + la_, :].T
        prm = np.zeros((128, 4), np.float32)
        prm[:, 0] = prms[0][g_]
        prm[:, 1] = prms[1][g_]
        prm[:, 2] = prms[2][g_]
        prm[:, 3] = prms[3][g_]
        in_maps.append({
            "f_in": f,
            "prm": prm,
            "GA": consts["GA"][g_],
            "BZ": consts["BZ"][g_],
            "CZ": consts["CZ"][g_],
            "GD": consts["GD"][g_],
            "kmA": consts["kmA"],
            "kmB": consts["kmB"],
            "wdA": consts["wdA"],
            "wdB": consts["wdB"],
        })
    return in_maps


def _assemble(outs):
    energy = np.zeros((C, NLAT, NLON), np.float32)
    for g_ in range(NCORES):
        oe = outs[g_]["out_e"]                  # (128, 64)
        for la_ in range(LAPC):
            energy[:, 4 * g_ + la_, :] = oe[32 * la_:32 * la_ + 8, :]
    pred = energy.reshape(C, NLAT * NLON).T.reshape(1, C, NLAT, NLON)
    return pred


def kernel(inp, amplitude, shift, beta, hypera, _trace=False,
           _trace_cores=None):
    from concourse.bass_utils import run_bass_kernel_spmd

    nc = _build_module()
    in_maps = _make_in_maps(inp, amplitude, shift, beta, hypera)
    res = run_bass_kernel_spmd(nc, in_maps, core_ids=list(range(NCORES)),
                               trace=_trace, trace_cores=_trace_cores)
    out = _assemble(res.results)
    if _trace:
        kernel.last_results = res
    return out
